# revision 35
# baseline (speedup 1.0000x reference)
"""MoE FFN (top-2 of 8 experts, SwiGLU) for 8 Trainium2 NeuronCores.

Strategy: expert parallelism. The gate (tiny: T x D @ D x E) plus top-2
routing runs on the host in float64; tokens are dispatched host-side to the
core owning their expert (one expert per core), each core runs the dense
SwiGLU FFN for its expert over its (capacity C) token batch, scales rows by
the combine weight on-device, and the host scatter-adds the two weighted
contributions per token.

Matmuls run in bf16 (PE streams 1 row/cycle at 2.4 GHz for <=2-byte dtypes
vs 2 cycles/row for fp32/fp32r) with fp32 PSUM accumulation; set
_MM_DTYPE = "f32r" for ~2.6e-4 rel err at half the PE rate (bf16 measures
~4e-3 abs/scale on this problem).

Device layouts are pre-permuted on the host so every DMA is a contiguous
128-partition transfer:
  xt  [D/128, 128, C]        xt[d, p, t]   = x_tokens[t, d*128+p]
  w1  [F/128, 128, 2, D/128, 128]  (f, p, uv, d, j) = W1[d*128+p, uv*F + f*128+j]
  w2  [F/128, 128, D]        w2[g, p, :]   = W2[g*128+p, :]
  wt  [128, C]               combine weight per token, replicated over partitions
  yt  [D/128, 128, C]        yt[d, p, t]   = y[t, d*128+p]  (weight-scaled)

Phase 1 computes hT = W1.T @ xT per 128-feature block ([feature, token]
layout, tokens moving), fuses silu(u)*v into on-chip bf16 `a` tiles; phase 2
computes yT = W2.T @ a with W2 column blocks stationary and tokens moving
(no 128-token quantization), scaling by wt on the PSUM->SBUF hop.
"""

import numpy as np

_B, _L, _D, _F, _E, _TOPK = 2, 2048, 1024, 2048, 8, 2

_MM_DTYPE = "bf16"  # "bf16" | "f32r"


def _split_multi_waits(nc):
    """The walrus build staged here allows at most ONE sem wait per
    instruction encoding ("Too many sync wait commands"). Split extra waits
    onto single-wait same-engine NOPs placed immediately before the
    instruction — semantically identical (the engine stalls on each in
    order)."""
    from concourse import mybir

    k = 0
    for f in nc.m.functions:
        for bb in f.blocks:
            insts = bb.instructions
            out = []
            for inst in insts:
                si = inst.sync_info
                waits = list(si.on_wait) if si is not None and si.on_wait else []
                if len(waits) > 1:
                    for w in waits[:-1]:
                        nop = mybir.InstNoOp(
                            name=f"waitsplit-{k}",
                            engine=inst.engine,
                            sync_info=mybir.SyncInfo(on_wait=[w], on_update=[]),
                            bass_nofuse=True,
                        )
                        k += 1
                        out.append(nop)
                    si.on_wait = [waits[-1]]
                out.append(inst)
            if k:
                bb.instructions = out


def _route(xf, Wg, expert_bias):
    """Top-2 routing in float64 (margin to the fp32 jax reference is ~50x the
    fp32 matmul noise for this problem's min top2/3rd gap)."""
    logits = xf.astype(np.float64) @ Wg.astype(np.float64) + expert_bias.astype(
        np.float64
    )
    order = np.argsort(-logits, axis=-1, kind="stable")
    idx = order[:, :_TOPK]  # [T, 2]
    l2 = np.take_along_axis(logits, idx, axis=-1)
    # renormalized top-2 softmax weights == softmax over the two top logits
    m = l2.max(axis=-1, keepdims=True)
    e = np.exp(l2 - m)
    w = (e / e.sum(axis=-1, keepdims=True)).astype(np.float32)  # [T, 2]
    counts = np.bincount(idx.ravel(), minlength=_E).astype(np.float32)
    return idx, w, counts


def _token_chunks(C):
    """Split C into phase-1 moving-dim chunks in [256, 512]; smaller only
    when C < 256."""
    chunks = []
    pos, rem = 0, C
    while rem > 0:
        if rem <= 512:
            take = rem
        elif rem >= 512 + 256:
            take = 512
        else:
            take = rem - 256
        chunks.append((pos, take))
        pos += take
        rem -= take
    return chunks


def _build_nc(C, mm_dtype=_MM_DTYPE):
    import concourse.bass as bass
    import concourse.tile as tile
    from concourse import mybir

    f32 = mybir.dt.float32
    mdt = mybir.dt.bfloat16 if mm_dtype == "bf16" else mybir.dt.float32r
    Silu = mybir.ActivationFunctionType.Silu

    KD = _D // 128  # 8
    KF = _F // 128  # 16
    chunks = _token_chunks(C)

    nc = bass.Bass()
    xt = nc.dram_tensor("xt", [KD, 128, C], mdt, kind="ExternalInput")
    w1 = nc.dram_tensor("w1", [KF, 128, 2, KD, 128], mdt, kind="ExternalInput")
    w2 = nc.dram_tensor("w2", [KF, 128, _D], mdt, kind="ExternalInput")
    wt = nc.dram_tensor("wt", [128, C], f32, kind="ExternalInput")
    y = nc.dram_tensor("yt", [KD, 128, C], f32, kind="ExternalOutput")

    from concourse.tile import add_dep_helper

    with tile.TileContext(nc) as tc:
        with (
            tc.tile_pool(name="xt", bufs=KD) as xt_pool,
            tc.tile_pool(name="w2r", bufs=KF) as w2_pool,
            tc.tile_pool(name="wt", bufs=1) as wt_pool,
            tc.tile_pool(name="a", bufs=KF) as a_pool,
            tc.tile_pool(name="w1s", bufs=4) as w1_pool,
            tc.tile_pool(name="s", bufs=4) as s_pool,
            tc.tile_pool(name="y", bufs=4) as y_pool,
            tc.tile_pool(name="psum", bufs=8, space="PSUM") as psum_pool,
        ):
            # Head critical path: w1[0] rides the otherwise-idle SWDGE path
            # (gpsimd) while xt[0] (split so chunk 0 leads) heads the SP
            # HWDGE FIFO; both first-matmul operands land during the
            # preamble window.
            w1_first = w1_pool.tile([128, 2, KD, 128], mdt, tag="w1s", name="w1b_0")
            nc.gpsimd.dma_start(w1_first[:], w1[0])
            xt_sb = []
            for d in range(KD):
                t = xt_pool.tile([128, C], mdt, tag="xt", name=f"xt_{d}")
                if d == 0 and len(chunks) > 1:
                    c1 = chunks[0][1]
                    nc.sync.dma_start(t[:, :c1], xt[d, :, :c1])
                    nc.sync.dma_start(t[:, c1:], xt[d, :, c1:])
                else:
                    nc.sync.dma_start(t[:], xt[d])
                xt_sb.append(t)
            wt_sb = wt_pool.tile([128, C], f32, tag="wt")
            nc.sync.dma_start(wt_sb[:], wt[:])
            # W2 resident (bf16: 32KB/partition). Loaded on the SWDGE ring
            # (gpsimd) and paced behind phase-1 progress via explicit dep
            # edges so the loads never compete with the critical head DMAs.
            w2_sb = [
                w2_pool.tile([128, _D], mdt, tag="w2r", name=f"w2_{g}")
                for g in range(KF)
            ]

            # ---- phase 1: h = x @ W1 ; a = silu(u) * v  (a on-chip, [f, tok]) ----
            a_tiles = []
            pace_insts = []  # last DVE mul of each f-iteration
            for f in range(KF):
                if f == 0:
                    w1blk = w1_first
                else:
                    w1blk = w1_pool.tile(
                        [128, 2, KD, 128], mdt, tag="w1s", name=f"w1b_{f}"
                    )
                    nc.sync.dma_start(w1blk[:], w1[f])
                w2dma = nc.gpsimd.dma_start(w2_sb[f][:], w2[f])
                if pace_insts:
                    add_dep_helper(
                        w2dma.ins,
                        pace_insts[-1].ins,
                        sync=True,
                        reason="pace resident w2 load behind phase 1",
                    )
                a_f = a_pool.tile([128, C], mdt, tag="a")
                pus = [
                    psum_pool.tile([128, cn], f32, tag="ps", name=f"pu_{f}_{ci}")
                    for ci, (_, cn) in enumerate(chunks)
                ]
                pvs = [
                    psum_pool.tile([128, cn], f32, tag="ps", name=f"pv_{f}_{ci}")
                    for ci, (_, cn) in enumerate(chunks)
                ]
                for uv, ps in ((0, pus), (1, pvs)):
                    for d in range(KD):
                        lhsT = w1blk[:, uv, d, :]
                        for ci, (c0, cn) in enumerate(chunks):
                            nc.tensor.matmul(
                                ps[ci][:],
                                lhsT,
                                xt_sb[d][:, c0 : c0 + cn],
                                start=(d == 0),
                                stop=(d == KD - 1),
                            )
                for ci, (c0, cn) in enumerate(chunks):
                    s = s_pool.tile([128, 512], f32, tag="s")
                    nc.scalar.activation(s[:, :cn], pus[ci][:], Silu)
                    mul = nc.vector.tensor_mul(
                        a_f[:, c0 : c0 + cn], s[:, :cn], pvs[ci][:]
                    )
                pace_insts.append(mul)
                a_tiles.append(a_f)

            # ---- phase 2: yT = W2.T @ a, scaled by wt along the token
            # (free) dim. Token-moving: no 128-token quantization, and the
            # yT[d] stores are large contiguous-row transfers. ----
            for m in range(KD):
                pys = [
                    psum_pool.tile([128, cn], f32, tag="ps", name=f"py_{m}_{ci}")
                    for ci, (_, cn) in enumerate(chunks)
                ]
                for g in range(KF):
                    lhsT = w2_sb[g][:, m * 128 : (m + 1) * 128]
                    for ci, (c0, cn) in enumerate(chunks):
                        nc.tensor.matmul(
                            pys[ci][:],
                            lhsT,
                            a_tiles[g][:, c0 : c0 + cn],
                            start=(g == 0),
                            stop=(g == KF - 1),
                        )
                ysb = y_pool.tile([128, C], f32, tag="y")
                for ci, (c0, cn) in enumerate(chunks):
                    nc.vector.tensor_mul(
                        ysb[:, c0 : c0 + cn], pys[ci][:], wt_sb[:, c0 : c0 + cn]
                    )
                if m == KD - 1 and len(chunks) > 1:
                    # Last tile gates the kernel tail: store it per-chunk so
                    # the final DMA is only the smallest (last) chunk.
                    for ci, (c0, cn) in enumerate(chunks):
                        nc.scalar.dma_start(
                            y[m, :, c0 : c0 + cn], ysb[:, c0 : c0 + cn]
                        )
                else:
                    nc.scalar.dma_start(y[m], ysb[:])

    return nc


def kernel(x, Wg, W1, W2, expert_bias):
    from concourse.bass_utils import run_bass_kernel_spmd

    x = np.asarray(x, dtype=np.float32)
    Wg = np.asarray(Wg, dtype=np.float32)
    W1 = np.asarray(W1, dtype=np.float32)
    W2 = np.asarray(W2, dtype=np.float32)
    expert_bias = np.asarray(expert_bias, dtype=np.float32)

    if _MM_DTYPE == "bf16":
        import ml_dtypes

        np_mdt = ml_dtypes.bfloat16
    else:
        np_mdt = np.float32

    T = _B * _L
    xf = x.reshape(T, _D)
    idx, w, counts = _route(xf, Wg, expert_bias)

    tok_lists = []
    wt_lists = []
    for e in range(_E):
        sel = idx == e  # [T, 2]
        toks = np.nonzero(sel.any(axis=-1))[0]
        wts = w[toks][sel[toks]]  # one hit per row: weight of expert e per token
        tok_lists.append(toks)
        wt_lists.append(wts.astype(np.float32))

    C = max(128, max(len(t) for t in tok_lists))
    KD, KF = _D // 128, _F // 128
    Cp = -(-C // 128) * 128  # host staging padded to full 128 tiles

    in_maps = []
    for e in range(_E):
        toks = tok_lists[e]
        n = len(toks)
        xg = np.zeros((Cp, _D), dtype=np.float32)
        xg[:n] = xf[toks]
        # [Cp, D] -> [KD, 128, C]: xt[d, p, t] = xg[t, d*128+p]
        xt_h = np.ascontiguousarray(
            xg.reshape(Cp, KD, 128).transpose(1, 2, 0)[:, :, :C]
        ).astype(np_mdt)
        w1_h = np.ascontiguousarray(
            W1[e].reshape(KD, 128, 2, KF, 128).transpose(3, 1, 2, 0, 4)
        ).astype(np_mdt)
        w2_h = np.ascontiguousarray(W2[e].reshape(KF, 128, _D)).astype(np_mdt)
        wt_full = np.zeros(C, dtype=np.float32)
        wt_full[:n] = wt_lists[e]
        wt_h = np.ascontiguousarray(np.broadcast_to(wt_full[None, :], (128, C)))
        in_maps.append({"xt": xt_h, "w1": w1_h, "w2": w2_h, "wt": wt_h})

    nc = _build_nc(C)
    _split_multi_waits(nc)
    res = run_bass_kernel_spmd(nc, in_maps, core_ids=list(range(_E)))

    out = np.zeros((T, _D), dtype=np.float32)
    for e in range(_E):
        toks = tok_lists[e]
        n = len(toks)
        if n:
            yt = res.results[e]["yt"].reshape(_D, C)  # yt[d, t] = y[t, d]
            out[toks] += yt[:, :n].T

    return out.reshape(_B, _L, _D), counts


# revision 36
# speedup vs baseline: 1.0212x; 1.0212x over previous
"""MoE FFN (top-2 of 8 experts, SwiGLU) for 8 Trainium2 NeuronCores.

Strategy: expert parallelism. The gate (tiny: T x D @ D x E) plus top-2
routing runs on the host in float64; tokens are dispatched host-side to the
core owning their expert (one expert per core), each core runs the dense
SwiGLU FFN for its expert over its (capacity C) token batch, scales rows by
the combine weight on-device, and the host scatter-adds the two weighted
contributions per token.

Matmuls run in bf16 (PE streams 1 row/cycle at 2.4 GHz for <=2-byte dtypes
vs 2 cycles/row for fp32/fp32r) with fp32 PSUM accumulation; set
_MM_DTYPE = "f32r" for ~2.6e-4 rel err at half the PE rate (bf16 measures
~4e-3 abs/scale on this problem).

Device layouts are pre-permuted on the host so every DMA is a contiguous
128-partition transfer:
  xt  [D/128, 128, C]        xt[d, p, t]   = x_tokens[t, d*128+p]
  w1  [F/128, 128, 2, D/128, 128]  (f, p, uv, d, j) = W1[d*128+p, uv*F + f*128+j]
  w2  [F/128, 128, D]        w2[g, p, :]   = W2[g*128+p, :]
  wt  [128, C]               combine weight per token, replicated over partitions
  yt  [D/128, 128, C]        yt[d, p, t]   = y[t, d*128+p]  (weight-scaled)

Phase 1 computes hT = W1.T @ xT per 128-feature block ([feature, token]
layout, tokens moving), fuses silu(u)*v into on-chip bf16 `a` tiles; phase 2
computes yT = W2.T @ a with W2 column blocks stationary and tokens moving
(no 128-token quantization), scaling by wt on the PSUM->SBUF hop.
"""

import numpy as np

_B, _L, _D, _F, _E, _TOPK = 2, 2048, 1024, 2048, 8, 2

_MM_DTYPE = "bf16"  # "bf16" | "f32r"


def _split_multi_waits(nc):
    """The walrus build staged here allows at most ONE sem wait per
    instruction encoding ("Too many sync wait commands"). Split extra waits
    onto single-wait same-engine NOPs placed immediately before the
    instruction — semantically identical (the engine stalls on each in
    order)."""
    from concourse import mybir

    k = 0
    for f in nc.m.functions:
        for bb in f.blocks:
            insts = bb.instructions
            out = []
            for inst in insts:
                si = inst.sync_info
                waits = list(si.on_wait) if si is not None and si.on_wait else []
                if len(waits) > 1:
                    for w in waits[:-1]:
                        nop = mybir.InstNoOp(
                            name=f"waitsplit-{k}",
                            engine=inst.engine,
                            sync_info=mybir.SyncInfo(on_wait=[w], on_update=[]),
                            bass_nofuse=True,
                        )
                        k += 1
                        out.append(nop)
                    si.on_wait = [waits[-1]]
                out.append(inst)
            if k:
                bb.instructions = out


def _route(xf, Wg, expert_bias):
    """Top-2 routing in float64 (margin to the fp32 jax reference is ~50x the
    fp32 matmul noise for this problem's min top2/3rd gap)."""
    logits = xf.astype(np.float64) @ Wg.astype(np.float64) + expert_bias.astype(
        np.float64
    )
    order = np.argsort(-logits, axis=-1, kind="stable")
    idx = order[:, :_TOPK]  # [T, 2]
    l2 = np.take_along_axis(logits, idx, axis=-1)
    # renormalized top-2 softmax weights == softmax over the two top logits
    m = l2.max(axis=-1, keepdims=True)
    e = np.exp(l2 - m)
    w = (e / e.sum(axis=-1, keepdims=True)).astype(np.float32)  # [T, 2]
    counts = np.bincount(idx.ravel(), minlength=_E).astype(np.float32)
    return idx, w, counts


def _token_chunks(C):
    """Split C into phase-1 moving-dim chunks in [256, 512]; smaller only
    when C < 256."""
    chunks = []
    pos, rem = 0, C
    while rem > 0:
        if rem <= 512:
            take = rem
        elif rem >= 512 + 256:
            take = 512
        else:
            take = rem - 256
        chunks.append((pos, take))
        pos += take
        rem -= take
    return chunks


def _build_nc(C, mm_dtype=_MM_DTYPE):
    import concourse.bass as bass
    import concourse.tile as tile
    from concourse import mybir

    f32 = mybir.dt.float32
    mdt = mybir.dt.bfloat16 if mm_dtype == "bf16" else mybir.dt.float32r
    Silu = mybir.ActivationFunctionType.Silu

    KD = _D // 128  # 8
    KF = _F // 128  # 16
    chunks = _token_chunks(C)

    nc = bass.Bass()
    xt = nc.dram_tensor("xt", [KD, 128, C], mdt, kind="ExternalInput")
    w1 = nc.dram_tensor("w1", [KF, 128, 2, KD, 128], mdt, kind="ExternalInput")
    w2 = nc.dram_tensor("w2", [KF, 128, _D], mdt, kind="ExternalInput")
    wt = nc.dram_tensor("wt", [128, C], f32, kind="ExternalInput")
    y = nc.dram_tensor("yt", [KD, 128, C], f32, kind="ExternalOutput")

    from concourse.tile import add_dep_helper

    with tile.TileContext(nc) as tc:
        with (
            tc.tile_pool(name="xt", bufs=KD) as xt_pool,
            tc.tile_pool(name="w2r", bufs=KF) as w2_pool,
            tc.tile_pool(name="wt", bufs=1) as wt_pool,
            tc.tile_pool(name="a", bufs=KF) as a_pool,
            tc.tile_pool(name="w1s", bufs=4) as w1_pool,
            tc.tile_pool(name="s", bufs=4) as s_pool,
            tc.tile_pool(name="y", bufs=4) as y_pool,
            tc.tile_pool(name="psum", bufs=8, space="PSUM") as psum_pool,
        ):
            # Head critical path: w1[0] rides the otherwise-idle SWDGE path
            # (gpsimd) while xt[0] (split so chunk 0 leads) heads the SP
            # HWDGE FIFO; both first-matmul operands land during the
            # preamble window.
            w1_first = w1_pool.tile([128, 2, KD, 128], mdt, tag="w1s", name="w1b_0")
            nc.gpsimd.dma_start(w1_first[:], w1[0])
            xt_sb = []
            for d in range(KD):
                t = xt_pool.tile([128, C], mdt, tag="xt", name=f"xt_{d}")
                if d == 0 and len(chunks) > 1:
                    c1 = chunks[0][1]
                    nc.sync.dma_start(t[:, :c1], xt[d, :, :c1])
                    nc.sync.dma_start(t[:, c1:], xt[d, :, c1:])
                else:
                    nc.sync.dma_start(t[:], xt[d])
                xt_sb.append(t)
            wt_sb = wt_pool.tile([128, C], f32, tag="wt")
            nc.sync.dma_start(wt_sb[:], wt[:])
            # W2 resident (bf16: 32KB/partition). Loaded on the SWDGE ring
            # (gpsimd) and paced behind phase-1 progress via explicit dep
            # edges so the loads never compete with the critical head DMAs.
            w2_sb = [
                w2_pool.tile([128, _D], mdt, tag="w2r", name=f"w2_{g}")
                for g in range(KF)
            ]

            # ---- phase 1: h = x @ W1 ; a = silu(u) * v  (a on-chip, [f, tok]) ----
            a_tiles = []
            pace_insts = []  # last DVE mul of each f-iteration
            for f in range(KF):
                if f == 0:
                    w1blk = w1_first
                else:
                    w1blk = w1_pool.tile(
                        [128, 2, KD, 128], mdt, tag="w1s", name=f"w1b_{f}"
                    )
                    nc.sync.dma_start(w1blk[:], w1[f])
                w2dma = nc.gpsimd.dma_start(w2_sb[f][:], w2[f])
                if pace_insts:
                    add_dep_helper(
                        w2dma.ins,
                        pace_insts[-1].ins,
                        sync=True,
                        reason="pace resident w2 load behind phase 1",
                    )
                a_f = a_pool.tile([128, C], mdt, tag="a")
                pus = [
                    psum_pool.tile([128, cn], f32, tag="ps", name=f"pu_{f}_{ci}")
                    for ci, (_, cn) in enumerate(chunks)
                ]
                pvs = [
                    psum_pool.tile([128, cn], f32, tag="ps", name=f"pv_{f}_{ci}")
                    for ci, (_, cn) in enumerate(chunks)
                ]
                # f=0 runs while the xt[d] tiles are still streaming in:
                # interleave u/v per d there so each xt tile feeds 6 matmuls
                # (~0.9us), matching the DMA arrival rate instead of
                # stalling the u-chain.
                if f == 0:
                    order = [(d, uv) for d in range(KD) for uv in (0, 1)]
                else:
                    order = [(d, uv) for uv in (0, 1) for d in range(KD)]
                for d, uv in order:
                    ps = pus if uv == 0 else pvs
                    lhsT = w1blk[:, uv, d, :]
                    for ci, (c0, cn) in enumerate(chunks):
                        nc.tensor.matmul(
                            ps[ci][:],
                            lhsT,
                            xt_sb[d][:, c0 : c0 + cn],
                            start=(d == 0),
                            stop=(d == KD - 1),
                        )
                for ci, (c0, cn) in enumerate(chunks):
                    s = s_pool.tile([128, 512], f32, tag="s")
                    nc.scalar.activation(s[:, :cn], pus[ci][:], Silu)
                    mul = nc.vector.tensor_mul(
                        a_f[:, c0 : c0 + cn], s[:, :cn], pvs[ci][:]
                    )
                pace_insts.append(mul)
                a_tiles.append(a_f)

            # ---- phase 2: yT = W2.T @ a, scaled by wt along the token
            # (free) dim. Token-moving: no 128-token quantization, and the
            # yT[d] stores are large contiguous-row transfers. ----
            for m in range(KD):
                pys = [
                    psum_pool.tile([128, cn], f32, tag="ps", name=f"py_{m}_{ci}")
                    for ci, (_, cn) in enumerate(chunks)
                ]
                for g in range(KF):
                    lhsT = w2_sb[g][:, m * 128 : (m + 1) * 128]
                    for ci, (c0, cn) in enumerate(chunks):
                        nc.tensor.matmul(
                            pys[ci][:],
                            lhsT,
                            a_tiles[g][:, c0 : c0 + cn],
                            start=(g == 0),
                            stop=(g == KF - 1),
                        )
                ysb = y_pool.tile([128, C], f32, tag="y")
                for ci, (c0, cn) in enumerate(chunks):
                    nc.vector.tensor_mul(
                        ysb[:, c0 : c0 + cn], pys[ci][:], wt_sb[:, c0 : c0 + cn]
                    )
                if m == KD - 1 and len(chunks) > 1:
                    # Last tile gates the kernel tail: store it per-chunk so
                    # the final DMA is only the smallest (last) chunk.
                    for ci, (c0, cn) in enumerate(chunks):
                        nc.scalar.dma_start(
                            y[m, :, c0 : c0 + cn], ysb[:, c0 : c0 + cn]
                        )
                else:
                    nc.scalar.dma_start(y[m], ysb[:])

    return nc


def kernel(x, Wg, W1, W2, expert_bias):
    from concourse.bass_utils import run_bass_kernel_spmd

    x = np.asarray(x, dtype=np.float32)
    Wg = np.asarray(Wg, dtype=np.float32)
    W1 = np.asarray(W1, dtype=np.float32)
    W2 = np.asarray(W2, dtype=np.float32)
    expert_bias = np.asarray(expert_bias, dtype=np.float32)

    if _MM_DTYPE == "bf16":
        import ml_dtypes

        np_mdt = ml_dtypes.bfloat16
    else:
        np_mdt = np.float32

    T = _B * _L
    xf = x.reshape(T, _D)
    idx, w, counts = _route(xf, Wg, expert_bias)

    tok_lists = []
    wt_lists = []
    for e in range(_E):
        sel = idx == e  # [T, 2]
        toks = np.nonzero(sel.any(axis=-1))[0]
        wts = w[toks][sel[toks]]  # one hit per row: weight of expert e per token
        tok_lists.append(toks)
        wt_lists.append(wts.astype(np.float32))

    C = max(128, max(len(t) for t in tok_lists))
    KD, KF = _D // 128, _F // 128
    Cp = -(-C // 128) * 128  # host staging padded to full 128 tiles

    in_maps = []
    for e in range(_E):
        toks = tok_lists[e]
        n = len(toks)
        xg = np.zeros((Cp, _D), dtype=np.float32)
        xg[:n] = xf[toks]
        # [Cp, D] -> [KD, 128, C]: xt[d, p, t] = xg[t, d*128+p]
        xt_h = np.ascontiguousarray(
            xg.reshape(Cp, KD, 128).transpose(1, 2, 0)[:, :, :C]
        ).astype(np_mdt)
        w1_h = np.ascontiguousarray(
            W1[e].reshape(KD, 128, 2, KF, 128).transpose(3, 1, 2, 0, 4)
        ).astype(np_mdt)
        w2_h = np.ascontiguousarray(W2[e].reshape(KF, 128, _D)).astype(np_mdt)
        wt_full = np.zeros(C, dtype=np.float32)
        wt_full[:n] = wt_lists[e]
        wt_h = np.ascontiguousarray(np.broadcast_to(wt_full[None, :], (128, C)))
        in_maps.append({"xt": xt_h, "w1": w1_h, "w2": w2_h, "wt": wt_h})

    nc = _build_nc(C)
    _split_multi_waits(nc)
    res = run_bass_kernel_spmd(nc, in_maps, core_ids=list(range(_E)))

    out = np.zeros((T, _D), dtype=np.float32)
    for e in range(_E):
        toks = tok_lists[e]
        n = len(toks)
        if n:
            yt = res.results[e]["yt"].reshape(_D, C)  # yt[d, t] = y[t, d]
            out[toks] += yt[:, :n].T

    return out.reshape(_B, _L, _D), counts
